# revision 13
# baseline (speedup 1.0000x reference)
"""BitNet transformer block on 8 Trainium2 NeuronCores (Bass/Tile SPMD).

v2: fp8e4m3 DoubleRow matmuls for all projections (ternary weights are exact
in fp8), rmsnorm scales folded into PSUM evicts (per-token rsq broadcast for
qkv; ln2's rsq^3 applied host-side on the down-proj output), software-
pipelined attention, o_proj/ln2/AllGather interleaved into attention-hi and
MLP-c0 emission, resident fp8 MLP weights, one bf16 ReduceScatter per
512-token chunk.

Sharding: head-parallel attention (core i owns q heads 2i,2i+1; kv head i//2)
with AllToAll to token-sharded o (core i owns blocks {i, 15-i}); o_proj/ln2
data-parallel over tokens + AllGather of fp8 mid activations; MLP
tensor-parallel over INTER/8 with token-chunked ReduceScatter.
"""

import sys

import numpy as np

try:
    import concourse.bass as bass  # noqa: F401
except Exception:  # pragma: no cover
    sys.path.insert(0, "/opt/trn_rl_repo")

import ml_dtypes
import concourse.bass as bass
import concourse.mybir as mybir
import concourse.tile as tile
from concourse import bacc
from concourse.bass_utils import run_bass_kernel_spmd

FP32 = mybir.dt.float32
BF16 = mybir.dt.bfloat16
F8 = mybir.dt.float8e4
BF = ml_dtypes.bfloat16
F8NP = ml_dtypes.float8_e4m3
DR = mybir.MatmulPerfMode.DoubleRow

ALPHA = 0.7
EPS = 1e-5
NH = 16          # query heads
NKV = 4          # kv heads
D = 128          # head dim
H = 2048         # hidden
I_TOT = 8192     # mlp intermediate
S = 2048         # sequence
NC = 8           # cores
P = 128
HT = H // P      # 16 hidden tiles
B = S // P       # 16 token blocks
I_LOC = I_TOT // NC   # 1024 intermediate per core
IT = I_LOC // P       # 8 inter tiles per core
TOK = 256             # tokens owned per core (2 blocks)
CH4 = 512             # phase-1 token chunk
CHM = 512             # MLP token chunk
PIPE = 3              # attention QK->AV software pipeline distance

# gathered token order: core i contributes blocks [i, 15-i]
PERM = []
for _i in range(NC):
    PERM += [_i, 15 - _i]
# MLP/RS token order: lo blocks 0..7 then hi blocks 15..8
PERM_DOWN = list(range(8)) + list(range(15, 7, -1))

_CACHE = {}


def _build_program():
    nc = bacc.Bacc("TRN2", target_bir_lowering=False, debug=False, num_devices=NC)
    AF = mybir.ActivationFunctionType
    ALU = mybir.AluOpType
    rg = [list(range(NC))]

    def dram_in(name, shape, dt=FP32):
        return nc.dram_tensor(name, shape, dt, kind="ExternalInput")

    xq_in = dram_in("xq", [P, HT, S], F8)             # fp8 x^T, all tokens
    xo_in = dram_in("xo", [P, HT, TOK], BF16)         # own-token residual
    cos_in = dram_in("cosf", [P, S], BF16)
    sin_in = dram_in("sinf", [P, S], BF16)
    wq_in = dram_in("wq", [P, 2, HT, P], F8)
    wk_in = dram_in("wk", [P, HT, P], F8)
    wv_in = dram_in("wv", [P, HT, P], F8)
    wo_in = dram_in("wo", [P, HT, HT, P], F8)
    wg_in = dram_in("wg", [P, IT, HT, P], F8)
    wu_in = dram_in("wu", [P, IT, HT, P], F8)
    wd_in = dram_in("wd", [P, IT, H], F8)
    aq_in = dram_in("aq", [P, 2])
    ak_in = dram_in("ak", [P, 1])
    av_in = dram_in("av", [P, 1])
    ao_in = dram_in("ao", [P, HT])
    sgu_in = dram_in("sgu", [P, IT])                  # 0.25*ag^2*au
    ad_in = dram_in("ad", [P, HT])                    # 4*ad
    rT_in = dram_in("rT", [P, P], BF16)
    tril_in = dram_in("tril2", [P, TOK], BF16)
    iden_in = dram_in("iden", [P, P], BF16)
    ones_f_in = dram_in("ones_f", [P, P])
    ones_b_in = dram_in("ones_b", [P, 1], BF16)
    eps_in = dram_in("epsv", [P, 1])

    xmidT = nc.dram_tensor("xmidT", [P, HT, TOK], FP32, kind="ExternalOutput")
    rsq2o = nc.dram_tensor("rsq2o", [1, TOK], FP32, kind="ExternalOutput")
    outc = [nc.dram_tensor(f"outc{c}", [TOK, w], BF16, kind="ExternalOutput")
            for c, w in enumerate([512, 512, 512, 256, 256])]

    a2a_lo_in = nc.dram_tensor("a2a_lo_in", [NC, P, 2, P], F8)
    a2a_lo_out = nc.dram_tensor("a2a_lo_out", [NC, P, 2, P], F8)
    a2a_hi_in = nc.dram_tensor("a2a_hi_in", [NC, P, 2, P], F8)
    a2a_hi_out = nc.dram_tensor("a2a_hi_out", [NC, P, 2, P], F8)
    h2_in = [nc.dram_tensor(f"h2_in_{h}", [P, HT, P], F8) for h in range(2)]
    h2_g = [nc.dram_tensor(f"h2_g_{h}", [NC * P, HT, P], F8, addr_space="Shared")
            for h in range(2)]
    MLP_CHUNKS = [(0, 0, 4), (0, 4, 4), (1, 0, 4), (1, 4, 2), (1, 6, 2)]
    rs_in = [nc.dram_tensor(f"rs_in_{c}", [H, 128 * nrk], BF16)
             for c, (_, _, nrk) in enumerate(MLP_CHUNKS)]
    rs_out = [nc.dram_tensor(f"rs_out_{c}", [TOK, 128 * nrk], BF16)
              for c, (_, _, nrk) in enumerate(MLP_CHUNKS)]

    with tile.TileContext(nc) as tc:
        # ---------------- persistent tiles ----------------
        const = tc.alloc_tile_pool(name="const", bufs=1)
        ones_f = const.tile([P, P], FP32)
        ones_b = const.tile([P, 1], BF16)
        rT = const.tile([P, P], BF16)
        iden = const.tile([P, P], BF16)
        tril2 = const.tile([P, TOK], BF16)
        cosf = const.tile([P, S], BF16)
        sinf = const.tile([P, S], BF16)
        aq = const.tile([P, 2], FP32)
        ak = const.tile([P, 1], FP32)
        av = const.tile([P, 1], FP32)
        ao = const.tile([P, HT], FP32)
        sgu = const.tile([P, IT], FP32)
        ad = const.tile([P, HT], FP32)
        eps_t = const.tile([P, 1], FP32)
        for dst, src in [(ones_f, ones_f_in), (ones_b, ones_b_in), (rT, rT_in),
                         (iden, iden_in), (tril2, tril_in), (eps_t, eps_in),
                         (cosf, cos_in), (sinf, sin_in),
                         (aq, aq_in), (ak, ak_in), (av, av_in), (ao, ao_in),
                         (sgu, sgu_in), (ad, ad_in)]:
            nc.scalar.dma_start(dst[:], src[:])

        wres = tc.alloc_tile_pool(name="wres", bufs=1)
        wq_sb = wres.tile([P, 2, HT, P], F8)
        wk_sb = wres.tile([P, HT, P], F8)
        wv_sb = wres.tile([P, HT, P], F8)
        nc.sync.dma_start(wq_sb[:], wq_in[:])
        nc.sync.dma_start(wk_sb[:], wk_in[:])
        nc.sync.dma_start(wv_sb[:], wv_in[:])

        wbig = tc.alloc_tile_pool(name="wbig", bufs=1)
        wo_all = wbig.tile([P, HT, HT, P], F8)
        wg_all = wbig.tile([P, IT, HT, P], F8)
        wu_all = wbig.tile([P, IT, HT, P], F8)
        wd_all = wbig.tile([P, IT, H], F8)

        actp = tc.alloc_tile_pool(name="actp", bufs=1)
        q_my = actp.tile([P, 2, S], BF16)
        k_my = actp.tile([P, B, P], BF16)
        v_my = actp.tile([P, B, P], BF16)
        o_my = actp.tile([P, HT, TOK], F8)
        xo = actp.tile([P, HT, TOK], BF16)
        nc.sync.dma_start(xo[:], xo_in[:])

        # ---------------- psum pool: 8 banks via 4 tagged rings ----------
        ps = tc.alloc_tile_pool(name="ps", bufs=1, space="PSUM")

        def psA(shape, dt=FP32, name="pa"):
            return ps.tile(shape, dt, name=name, tag="a", bufs=2)

        def psSP(shape, name="sps"):
            return ps.tile(shape, FP32, name=name, tag="s", bufs=PIPE)

        def psOP(shape, name="po"):
            return ps.tile(shape, FP32, name=name, tag="o", bufs=2)

        def psL(shape, name="pl"):
            return ps.tile(shape, FP32, name=name, tag="l", bufs=1)

        # ---------------- sbuf working pools (tagged rings) ----------------
        sb = tc.alloc_tile_pool(name="sb", bufs=1)       # whole-program tags
        sbP1 = tc.alloc_tile_pool(name="sbP1", bufs=1)   # phase-1 only tags

        def sbt(shape, dt, tag, bufs, name):
            return sb.tile(shape, dt, name=name, tag=tag, bufs=bufs)

        def sbt1(shape, dt, tag, bufs, name):
            return sbP1.tile(shape, dt, name=name, tag=tag, bufs=bufs)

        sbM_holder = []

        def sbtM(shape, dt, tag, bufs, name):
            return sbM_holder[0].tile(shape, dt, name=name, tag=tag, bufs=bufs)

        xc0 = sbt1([P, HT, CH4], F8, "xc", 2, "xc0")
        nc.sync.dma_start(xc0[:], xq_in[:, :, 0:CH4])

        # ================= phase 1: ln1 + qkv (DR fp8) ==================
        def p1_chunk(c4, xc=None):
            tsl = slice(c4 * CH4, (c4 + 1) * CH4)
            if xc is None:
                xc = sbt1([P, HT, CH4], F8, "xc", 2, "xc")
                nc.sync.dma_start(xc[:], xq_in[:, :, tsl])
            ssq = psA([1, CH4], name="ssq")
            for kt in range(HT):
                sqv = sbt1([P, CH4], BF16, "sqv", 2, "sqv")
                nc.vector.tensor_mul(sqv[:], xc[:, kt, :], xc[:, kt, :])
                nc.tensor.matmul(ssq[:], ones_b[:], sqv[:],
                                 start=(kt == 0), stop=(kt == HT - 1))
            ms = sbt([1, CH4], FP32, "nrm", 3, "ms")
            nc.scalar.activation(ms[:], ssq[:], AF.Identity, bias=eps_t[0:1, :],
                                 scale=1.0 / H)
            rec = sbt([1, CH4], FP32, "nrm", 3, "rec")
            nc.vector.reciprocal_approx_fast(rec[:], ms[:])
            rsq = sbt([1, CH4], FP32, "nrm", 3, "rsq")
            nc.scalar.activation(rsq[:], rec[:], AF.Sqrt)
            bc = psA([P, CH4], name="bc")
            nc.tensor.matmul(bc[:], ones_f[0:1, :], rsq[:], start=True, stop=True)
            bcs = sbt([P, CH4], FP32, "bcs", 2, "bcs")
            nc.scalar.activation(bcs[:], bc[:], AF.Copy)
            cfc = cosf[:, tsl]
            sfc = sinf[:, tsl]

            def proj(lhsT):
                pp = psA([P, CH4], name="pps")
                for j in range(HT // 2):
                    nc.tensor.matmul(pp[:], lhsT[:, 2 * j:2 * j + 2, :],
                                     xc[:, 2 * j:2 * j + 2, :],
                                     start=(j == 0), stop=(j == HT // 2 - 1),
                                     perf_mode=DR)
                return pp

            def rope_out(pp, scal, dst):
                qs = sbt1([P, CH4], BF16, "rope", 3, "qs")
                nc.vector.scalar_tensor_tensor(qs[:], pp[:], scal, bcs[:],
                                               ALU.mult, ALU.mult)
                rot = psA([P, CH4], name="rot")
                nc.tensor.matmul(rot[:], rT[:], qs[:], start=True, stop=True)
                t1 = sbt1([P, CH4], BF16, "rope", 3, "t1")
                nc.vector.tensor_mul(t1[:], rot[:], sfc)
                t2 = sbt1([P, CH4], BF16, "rope", 3, "t2")
                nc.vector.tensor_mul(t2[:], qs[:], cfc)
                nc.vector.tensor_add(dst, t1[:], t2[:])

            for f in range(2):
                rope_out(proj(wq_sb[:, f]), aq[:, f:f + 1], q_my[:, f, tsl])
            rope_out(proj(wk_sb), ak[:, 0:1],
                     k_my[:, 4 * c4:4 * c4 + 4, :].rearrange("p b t -> p (b t)"))
            pv = proj(wv_sb)
            vtv = sbt1([P, CH4], BF16, "rope", 3, "vtv")
            nc.vector.scalar_tensor_tensor(vtv[:], pv[:], av[:, 0:1], bcs[:],
                                           ALU.mult, ALU.mult)
            for j in range(4):
                vtp = psA([P, P], BF16, name="vtp")
                nc.tensor.transpose(vtp[:], vtv[:, j * P:(j + 1) * P], iden[:])
                nc.vector.tensor_copy(v_my[:, 4 * c4 + j, :], vtp[:])

        # ================= attention block (pipelined) ==================
        def attn_block(qb, finish_prev):
            state = {"ops": None, "lps": None}
            qv = q_my[:, :, qb * P:(qb + 1) * P]
            pm_q = {}

            def drain(kb):
                pm = pm_q.pop(kb)
                if state["lps"] is None:
                    state["lps"] = psL([1, TOK], name="lps")
                    state["ops"] = psOP([P, TOK], name="ops")
                nc.tensor.matmul(state["lps"][:], ones_b[:], pm,
                                 start=(kb == 0), stop=(kb == qb))
                nc.tensor.matmul(state["ops"][:], v_my[:, kb, :], pm,
                                 start=(kb == 0), stop=(kb == qb))

            for kb in range(qb + 1):
                sps = psSP([P, TOK])
                nc.tensor.matmul(sps[:], k_my[:, kb, :], qv,
                                 start=True, stop=True)
                pm = sbt([P, TOK], BF16, "pm", PIPE + 3, "pm")
                nc.scalar.activation(pm[:], sps[:], AF.Exp)
                if kb == qb:
                    pmm = sbt([P, TOK], BF16, "pm", PIPE + 3, "pmm")
                    nc.vector.tensor_mul(pmm[:], pm[:], tril2[:])
                    pm_q[kb] = pmm[:]
                else:
                    pm_q[kb] = pm[:]
                if kb == 1 and finish_prev is not None:
                    finish_prev()
                if kb >= PIPE:
                    drain(kb - PIPE)
            for kb in range(max(0, qb + 1 - PIPE), qb + 1):
                drain(kb)

            def finish():
                lsb = sbt([1, TOK], FP32, "fl", 3, "lsb")
                nc.scalar.activation(lsb[:], state["lps"][:], AF.Copy,
                                     scale=1.0 / 1.0625)
                linv = sbt([1, TOK], FP32, "fl", 3, "linv")
                nc.vector.reciprocal_approx_fast(linv[:], lsb[:])
                bca = psL([P, TOK], name="bca")
                nc.tensor.matmul(bca[:], ones_f[0:1, :], linv[:],
                                 start=True, stop=True)
                bcs_a = sbt([P, TOK], FP32, "bcsa", 2, "bcs_a")
                nc.scalar.activation(bcs_a[:], bca[:], AF.Copy)
                osb = sbt([P, TOK], F8, "osb", 2, "osb")
                nc.vector.tensor_mul(osb[:], state["ops"][:], bcs_a[:])
                r_dst = min(qb, 15 - qb)
                dst = a2a_lo_in if qb < 8 else a2a_hi_in
                nc.sync.dma_start(
                    dst[r_dst][:], osb[:].rearrange("p (h t) -> p h t", h=2))
            return finish

        # ================= o_proj + ln2 + AG for one half ==================
        def oproj_half(half, fr):
            csl = slice(half * P, (half + 1) * P)
            xm = xmid_h[half]
            for f in fr:
                pp = psA([P, P], name="ops5")
                for j in range(HT // 2):
                    nc.tensor.matmul(pp[:], wo_all[:, f, 2 * j:2 * j + 2, :],
                                     o_my[:, 2 * j:2 * j + 2, csl],
                                     start=(j == 0), stop=(j == HT // 2 - 1),
                                     perf_mode=DR)
                nc.vector.scalar_tensor_tensor(
                    xm[:, f, :], pp[:], ao[:, f:f + 1], xo[:, f, csl],
                    ALU.mult, ALU.add)

        def ln2_half(half):
            csl = slice(half * P, (half + 1) * P)
            xm = xmid_h[half]
            ssq2 = psA([1, P], name="ssq2")
            for kt in range(HT):
                sqv2 = sbt([P, P], BF16, "sqv2", 2, "sqv2")
                nc.vector.tensor_mul(sqv2[:], xm[:, kt, :], xm[:, kt, :])
                nc.tensor.matmul(ssq2[:], ones_b[:], sqv2[:],
                                 start=(kt == 0), stop=(kt == HT - 1))
            ms2 = sbt([1, P], FP32, "nrm", 3, "ms2")
            nc.scalar.activation(ms2[:], ssq2[:], AF.Identity,
                                 bias=eps_t[0:1, :], scale=1.0 / H)
            rec2 = sbt([1, P], FP32, "nrm", 3, "rec2")
            nc.vector.reciprocal_approx_fast(rec2[:], ms2[:])
            rsq2 = sbt([1, P], FP32, "nrm", 3, "rsq2")
            nc.scalar.activation(rsq2[:], rec2[:], AF.Sqrt)
            nc.sync.dma_start(rsq2o[:, csl], rsq2[:])
            h8 = sbt([P, HT, P], F8, "h8", 2, "h8")
            nc.scalar.activation(h8[:], xm[:], AF.Copy, scale=1.0625)
            nc.sync.dma_start(h2_in[half][:], h8[:])
            nc.sync.dma_start(xmidT[:, :, csl], xm[:])
            nc.gpsimd.collective_compute(
                "AllGather", ALU.bypass, ins=[h2_in[half][:]],
                outs=[h2_g[half][:]], replica_groups=rg)

        # ================= MLP chunk ==================
        h2lov = h2_g[0][:].rearrange("(r p) kt t -> r p kt t", r=NC)
        h2hiv = h2_g[1][:].rearrange("(r p) kt t -> r p kt t", r=NC)

        def mlp_chunk(c, interleave=None):
            half, rbase, nrk = MLP_CHUNKS[c]
            W = 128 * nrk
            h2v = h2lov if half == 0 else h2hiv
            h2c = sbtM([P, HT, W], F8, "h2c", 2, "h2c")
            for j in range(nrk):
                nc.scalar.dma_start(h2c[:, :, j * P:(j + 1) * P], h2v[rbase + j])
            m_all = sbtM([P, IT, W], F8, "mall", 2, "m_all")
            for f in range(IT):
                gps = psA([P, W], name="gps")
                for j in range(HT // 2):
                    nc.tensor.matmul(gps[:], wg_all[:, f, 2 * j:2 * j + 2, :],
                                     h2c[:, 2 * j:2 * j + 2, :],
                                     start=(j == 0), stop=(j == HT // 2 - 1),
                                     perf_mode=DR)
                ups = psA([P, W], name="ups")
                for j in range(HT // 2):
                    nc.tensor.matmul(ups[:], wu_all[:, f, 2 * j:2 * j + 2, :],
                                     h2c[:, 2 * j:2 * j + 2, :],
                                     start=(j == 0), stop=(j == HT // 2 - 1),
                                     perf_mode=DR)
                gr = sbtM([P, W], BF16, "gr", 2, "gr")
                nc.scalar.activation(gr[:], gps[:], AF.Relu)
                r2 = sbtM([P, W], BF16, "r2", 2, "r2")
                nc.vector.tensor_mul(r2[:], gr[:], gr[:])
                nc.vector.scalar_tensor_tensor(
                    m_all[:, f, :], ups[:], sgu[:, f:f + 1], r2[:],
                    ALU.mult, ALU.mult)
                if interleave is not None:
                    interleave(f)
            rs_iv = rs_in[c][:].rearrange("(f p) t -> f p t", p=P)
            for f in range(HT):
                dps = psA([P, W], name="dps")
                for j in range(IT // 2):
                    nc.tensor.matmul(dps[:], wd_all[:, 2 * j:2 * j + 2,
                                                    f * P:(f + 1) * P],
                                     m_all[:, 2 * j:2 * j + 2, :],
                                     start=(j == 0), stop=(j == IT // 2 - 1),
                                     perf_mode=DR)
                dn = sbtM([P, W], BF16, "dn", 3, "dn")
                nc.vector.tensor_scalar_mul(dn[:], dps[:], ad[:, f:f + 1])
                eng = nc.sync if f % 2 == 0 else nc.gpsimd
                eng.dma_start(rs_iv[f], dn[:])
            nc.gpsimd.collective_compute(
                "ReduceScatter", ALU.add, ins=[rs_in[c][:]],
                outs=[rs_out[c][:]], replica_groups=rg)
            nc.sync.dma_start(outc[c][:], rs_out[c][:])

        # ================= emission schedule ==================
        xmid_h = [None, None]

        with nc.named_scope("p1c0"):
            p1_chunk(0, xc=xc0)
        nc.gpsimd.dma_start(wo_all[:], wo_in[:])
        nc.gpsimd.dma_start(wg_all[:], wg_in[:])
        nc.gpsimd.dma_start(wu_all[:], wu_in[:])
        nc.gpsimd.dma_start(wd_all[:], wd_in[:])
        fin = None
        with nc.named_scope("attn03"):
            for qb in range(4):
                fin = attn_block(qb, fin)
        with nc.named_scope("p1c1"):
            p1_chunk(1)
        with nc.named_scope("attn47"):
            for qb in range(4, 8):
                fin = attn_block(qb, fin)
        fin()  # finish qb=7 now so A2A-lo can issue
        fin = None
        nc.gpsimd.collective_compute(
            "AllToAll", ALU.bypass, ins=[a2a_lo_in[:]],
            outs=[a2a_lo_out[:]], replica_groups=rg)
        with nc.named_scope("p1c2"):
            p1_chunk(2)
        for j in range(NC):
            nc.sync.dma_start(o_my[:, 2 * j:2 * j + 2, 0:P], a2a_lo_out[j])
        with nc.named_scope("attn811"):
            for qb in range(8, 12):
                fin = attn_block(qb, fin)
        with nc.named_scope("p1c3"):
            p1_chunk(3)
        xmid_h[0] = sbt([P, HT, P], FP32, "xm", 2, "xmh0")
        with nc.named_scope("tail_attn"):
            oproj_half(0, range(0, 8))
            fin = attn_block(12, fin)
            oproj_half(0, range(8, 16))
            fin = attn_block(13, fin)
            ln2_half(0)
            fin = attn_block(14, fin)
            fin = attn_block(15, fin)
        fin()  # finish qb=15 so A2A-hi can issue
        nc.gpsimd.collective_compute(
            "AllToAll", ALU.bypass, ins=[a2a_hi_in[:]],
            outs=[a2a_hi_out[:]], replica_groups=rg)
        for j in range(NC):
            nc.sync.dma_start(o_my[:, 2 * j:2 * j + 2, P:TOK], a2a_hi_out[j])
        sbP1.release()
        sbM_holder.append(tc.alloc_tile_pool(name="sbM", bufs=1))

        def inter_c0(f):
            if f == 5:
                xmid_h[1] = sbt([P, HT, P], FP32, "xm", 2, "xmh1")
                oproj_half(1, range(0, 8))
            elif f == 6:
                oproj_half(1, range(8, 16))
            elif f == 7:
                ln2_half(1)

        with nc.named_scope("mlp0"):
            mlp_chunk(0, interleave=inter_c0)
        with nc.named_scope("mlp1"):
            mlp_chunk(1)
        with nc.named_scope("mlp2"):
            mlp_chunk(2)
        with nc.named_scope("mlp34"):
            mlp_chunk(3)
            mlp_chunk(4)

        for pool in [sbM_holder[0], sb, ps, actp, wbig, wres, const]:
            pool.release()

    nc.finalize()
    return nc


def _ternary(w, fold_row=None):
    w = np.asarray(w, dtype=np.float32)
    am = np.mean(np.abs(w), axis=1)
    t = np.sign(w) * (np.abs(w) > ALPHA * am[:, None]).astype(np.float32)
    if fold_row is not None:
        t = t * fold_row[None, :]
    return t, am


def _wlhsT(tern, n_f):
    """ternary [O, Hin] -> lhsT layout [f, p, kt, c] fp8."""
    o, hin = tern.shape
    kt = hin // P
    assert n_f * P == o
    wT = np.ascontiguousarray(tern.T)
    return np.ascontiguousarray(
        wT.reshape(kt, P, n_f, P).transpose(2, 1, 0, 3)).astype(F8NP)


def _scale_tiles(a):
    return np.ascontiguousarray(a.reshape(-1, P).T).astype(np.float32)


def _pcol(x2d, dt=np.float32):
    k, t = x2d.shape
    return np.ascontiguousarray(
        x2d.reshape(k // P, P, t).transpose(1, 0, 2)).astype(dt)


def kernel(x, cos, sin, wq, wk, wv, wo, wg, wu, wd, ln1_w, ln2_w):
    x = np.asarray(x, dtype=np.float32)
    b, s, hdim = x.shape
    assert (b, s, hdim) == (1, S, H)

    if "nc" not in _CACHE:
        _CACHE["nc"] = _build_program()
    nc = _CACHE["nc"]

    ln1 = np.asarray(ln1_w, dtype=np.float32)
    ln2 = np.asarray(ln2_w, dtype=np.float32)

    tq, amq = _ternary(wq, fold_row=ln1)
    tk, amk = _ternary(wk, fold_row=ln1)
    tv, amv = _ternary(wv, fold_row=ln1)
    to, amo = _ternary(wo)
    tg, amg = _ternary(wg, fold_row=ln2)
    tu, amu = _ternary(wu, fold_row=ln2)
    td, amd = _ternary(wd)

    wq_h = _wlhsT(tq, NH)        # [16, P, HT, P]
    wk_h = _wlhsT(tk, NKV)
    wv_h = _wlhsT(tv, NKV)
    wo_h = np.ascontiguousarray(
        _wlhsT(to, HT).transpose(1, 0, 2, 3))        # [P, f, kt, c]
    wg_h = _wlhsT(tg, I_TOT // P)
    wu_h = _wlhsT(tu, I_TOT // P)
    wd_h = np.ascontiguousarray(
        td.T.reshape(I_TOT // P, P, H).transpose(1, 0, 2)).astype(F8NP)

    aq_h = _scale_tiles(amq / np.sqrt(np.float32(D)))
    ak_h = _scale_tiles(amk)
    av_h = _scale_tiles(amv)
    ao_h = _scale_tiles(amo)
    ag_h = _scale_tiles(amg)
    au_h = _scale_tiles(amu)
    ad_h = _scale_tiles(amd)

    x2 = x[0]
    xT = np.ascontiguousarray(x2.T)
    xq_h = _pcol(xT, F8NP)
    cosT = np.ascontiguousarray(np.asarray(cos, np.float32)[0, 0].T).astype(BF)
    sinT = np.ascontiguousarray(np.asarray(sin, np.float32)[0, 0].T).astype(BF)

    R = np.zeros((P, P), np.float32)
    for m in range(64):
        R[m, m + 64] = -1.0
        R[m + 64, m] = 1.0
    rT_h = np.ascontiguousarray(R.T).astype(BF)
    ones_f = np.ones((P, P), np.float32)
    ones_b = np.ones((P, 1), np.float32).astype(BF)
    triu = np.triu(np.ones((P, P), np.float32))
    tril2_h = np.ascontiguousarray(np.concatenate([triu, triu], axis=1)).astype(BF)
    iden_h = np.eye(P, dtype=np.float32).astype(BF)

    in_maps = []
    for i in range(NC):
        blo, bhi = i, 15 - i
        own_cols = np.r_[blo * P:(blo + 1) * P, bhi * P:(bhi + 1) * P]
        kvh = i // 2
        islice = slice(i * IT, (i + 1) * IT)
        ag_i = ag_h[:, islice]
        au_i = au_h[:, islice]
        in_maps.append({
            "xq": xq_h,
            "xo": _pcol(xT[:, own_cols], BF),
            "cosf": cosT, "sinf": sinT,
            "wq": np.ascontiguousarray(wq_h[2 * i:2 * i + 2].transpose(1, 0, 2, 3)),
            "wk": np.ascontiguousarray(wk_h[kvh]),
            "wv": np.ascontiguousarray(wv_h[kvh]),
            "wo": wo_h,
            "wg": np.ascontiguousarray(wg_h[islice].transpose(1, 0, 2, 3)),
            "wu": np.ascontiguousarray(wu_h[islice].transpose(1, 0, 2, 3)),
            "wd": np.ascontiguousarray(wd_h[:, islice, :]),
            "aq": np.ascontiguousarray(aq_h[:, 2 * i:2 * i + 2]),
            "ak": np.ascontiguousarray(ak_h[:, kvh:kvh + 1]),
            "av": np.ascontiguousarray(av_h[:, kvh:kvh + 1]),
            "ao": ao_h / np.float32(1.0625),
            "sgu": np.ascontiguousarray(
                0.25 * ag_i * ag_i * au_i / (1.0625 ** 2)),
            "ad": np.ascontiguousarray(4.0 * ad_h / 1.0625),
            "rT": rT_h, "tril2": tril2_h, "iden": iden_h,
            "ones_f": ones_f, "ones_b": ones_b,
            "epsv": np.full((P, 1), EPS, np.float32),
        })

    res = run_bass_kernel_spmd(nc, in_maps, list(range(NC)))
    _CACHE["last_result"] = res

    down_T = np.concatenate(
        [np.concatenate([res.results[i][f"outc{c}"] for c in range(5)],
                        axis=1).astype(np.float32)
         for i in range(NC)], axis=0)                     # [H, S] PERM_DOWN order
    xmid_T = np.concatenate(
        [res.results[i]["xmidT"].transpose(1, 0, 2).reshape(H, TOK)
         for i in range(NC)], axis=1)                     # [H, S] PERM order
    rsq2 = np.concatenate(
        [res.results[i]["rsq2o"][0] for i in range(NC)])  # [S] PERM order
    rsq_blk = {}
    for j, blk in enumerate(PERM):
        rsq_blk[blk] = rsq2[j * P:(j + 1) * P]

    out_T = np.empty_like(down_T)
    for j, blk in enumerate(PERM_DOWN):
        r3 = rsq_blk[blk] ** 3
        out_T[:, blk * P:(blk + 1) * P] = (
            down_T[:, j * P:(j + 1) * P] * r3[None, :])
    for j, blk in enumerate(PERM):
        out_T[:, blk * P:(blk + 1) * P] += xmid_T[:, j * P:(j + 1) * P]
    return np.ascontiguousarray(out_T.T).reshape(1, S, H).astype(np.float32)


if __name__ == "__main__":
    nc = _build_program()
    print("build OK; instructions:",
          sum(len(b.instructions) for f in nc.m.functions for b in f.blocks))
